# revision 55
# baseline (speedup 1.0000x reference)
"""Trainium2 Bass kernel for nn_OA_Layer (offset-attention layer).

Reference (per batch b, C=256, N=4096, CQK=64):
    xs = x + xyz
    q = k = wqk @ xs + bqk          [64, N]
    v = wv @ xs + bv                [C, N]
    E = q^T q                       [N, N]  (symmetric, since q == k)
    attn = softmax(E, rows) ; attn /= (1e-9 + attn.sum(rows))
    x_r = v @ attn
    t = wt @ (xs - x_r) + bt ; t = BN(t) ; x_r = leaky_relu(t, 0.2)
    out = xs + x_r

Sharding: data-parallel over batch B=8 across 8 cores (1 batch/core).

Math restructuring (exact up to fp rounding):
  - softmax row-shifted by diag d[n] = ||q_n||^2 (overflow safety);
    d computed directly in [128, NB] layout via N=1 matmuls
    (sq_chunk^T @ ones64)
  - rowsum rs[n] via fused Exp+accum in pass 1 (fp32)
  - pass-2 exp bias folds the row normalization: exp(E - d - ln rs),
    so a2 tiles are already row-normalized attn
  - colsum'[m] = sum_n a2[n,m] via ones-weighted matmuls (PSUM-accumulated)
  - invcs = exp(-ln(colsum + 1e-9)) on the ACT engine (cheap on [1,512])
  - x_r = (v @ a2) * invcs[m]; bv folded into bt' = bt - wt @ bv on host
  - BN+bias folded to t*g + bp_eff on host.

All matmul operands are bf16 (fast weight loads; host-simulated rel err
8.4e-4 vs the 2e-2 budget); PSUM accumulation and softmax statistics
stay fp32.

Scheduling: pass-2 runs row-block PAIRS (qrow 0/64 kpack halves execute
concurrently on disjoint PE row groups), pipelined one pair ahead of the
accumulation matmuls.  Chunk tails (invcs, y, t-conv, output) are spread
across the next chunk's pair loop; xr accumulators alternate PSUM banks
between chunks so tails are fully off the accumulation critical path.
PSUM: e2 strips 2 + xr 2x2 + tps 1 + cs 1 = 8 banks.
"""

import numpy as np

import concourse.bass as bass
import concourse.tile as tile
from concourse import bacc, mybir
from concourse._compat import with_exitstack

F32 = mybir.dt.float32
F32R = mybir.dt.float32r
BF16 = mybir.dt.bfloat16
F8 = mybir.dt.float8e4

C = 256
CQK = 64
P = 128
BN_EPS = 1e-5


def build_kernel(N=4096, debug=False):
    """Builds the per-core bass program. Returns nc."""
    nc = bacc.Bacc("TRN2", target_bir_lowering=False, debug=debug,
                   num_devices=8)

    x_d = nc.declare_dram_parameter("x", [C, N], F32, isOutput=False)
    xyz_d = nc.declare_dram_parameter("xyz", [C, N], F32, isOutput=False)
    wqkT_d = nc.declare_dram_parameter("wqkT", [C, CQK], BF16, isOutput=False)
    wvT_d = nc.declare_dram_parameter("wvT", [C, C], BF16, isOutput=False)
    wtT_d = nc.declare_dram_parameter("wtT", [C, C], BF16, isOutput=False)
    bqk_d = nc.declare_dram_parameter("bqk", [CQK, 1], F32, isOutput=False)
    g_d = nc.declare_dram_parameter("g", [C, 1], F32, isOutput=False)
    bp_d = nc.declare_dram_parameter("bp", [C, 1], F32, isOutput=False)
    out_d = nc.declare_dram_parameter("out", [C, N], F32, isOutput=True)

    with tile.TileContext(nc) as tc:
        _emit(nc, tc, N, x_d, xyz_d, wqkT_d, wvT_d, wtT_d, bqk_d, g_d, bp_d,
              out_d)
    nc.compile()
    return nc


@with_exitstack
def _emit(ctx, nc, tc, N,
          x_d, xyz_d, wqkT_d, wvT_d, wtT_d, bqk_d, g_d, bp_d, out_d):
    NB = N // P          # 32 row-blocks of 128
    MC = N // 512        # 8 column chunks of 512
    NPAIR = NB // 2      # 16 row-block pairs
    ek = ctx.enter_context

    consts = ek(tc.tile_pool(name="consts", bufs=1))
    big = ek(tc.tile_pool(name="big", bufs=1))
    stats = ek(tc.tile_pool(name="stats", bufs=1))

    # ---- constant / resident tensors (small loads on the gpsimd queue) ----
    wqkT = consts.tile([P, 2 * CQK], BF16)      # [p, (khalf, o)]
    nc.gpsimd.dma_start(wqkT[:].rearrange("p (t m) -> p t m", t=2),
                        wqkT_d[:].rearrange("(t p) m -> p t m", p=P))
    wvT = consts.tile([P, 2 * C], BF16)
    nc.gpsimd.dma_start(wvT[:].rearrange("p (t m) -> p t m", t=2),
                        wvT_d[:].rearrange("(t p) m -> p t m", p=P))
    wtT = consts.tile([P, 2 * C], BF16)
    nc.gpsimd.dma_start(wtT[:].rearrange("p (t m) -> p t m", t=2),
                        wtT_d[:].rearrange("(t p) m -> p t m", p=P))
    bqk = consts.tile([CQK, 1], F32)
    nc.gpsimd.dma_start(bqk[:], bqk_d[:])
    g_t = consts.tile([P, 2], F32)
    bp_t = consts.tile([P, 2], F32)
    for h in range(2):
        nc.gpsimd.dma_start(g_t[:, h:h + 1], g_d[h * P:(h + 1) * P, :])
        nc.gpsimd.dma_start(bp_t[:, h:h + 1], bp_d[h * P:(h + 1) * P, :])
    ones64 = consts.tile([CQK, 1], BF16)
    nc.vector.memset(ones64[:], 1.0)
    ones128 = consts.tile([P, 1], BF16)
    nc.vector.memset(ones128[:], 1.0)
    ones_row_f = consts.tile([1, P], F32)
    nc.vector.memset(ones_row_f[:], 1.0)
    ones_row = consts.tile([1, P], F32R)
    nc.vector.tensor_copy(ones_row[:], ones_row_f[:])
    ones_col_f = consts.tile([P, 1], F32)
    nc.vector.memset(ones_col_f[:], 1.0)
    ones_col = consts.tile([P, 1], F32R)
    nc.vector.tensor_copy(ones_col[:], ones_col_f[:])
    eps_t = consts.tile([1, 1], F32)
    nc.vector.memset(eps_t[:], 1e-9)

    # xs = x + xyz stored ONLY in bf16 (residual + matmul operand; the
    # extra 0.2% rounding costs ~1e-3 end-to-end, budget is 2e-2)
    # layout [128, 2*N]: c-half h at cols [h*N, (h+1)*N)
    xs_b = big.tile([P, 2 * N], BF16)
    q2 = big.tile([P, N], BF16)           # q duplicated on partition halves
    vT = big.tile([P, NB * C], BF16)      # v^T tile i at cols [i*C, (i+1)*C)
    negdiag = stats.tile([P, NB], F32)
    rs_acc = stats.tile([P, 2 * NB], F32)
    bias2 = stats.tile([P, NB], F32)

    zpool = ek(tc.tile_pool(name="zpool", bufs=2))
    with (
        tc.tile_pool(name="prepps", bufs=2, space=bass.MemorySpace.PSUM)
        as prepps,
        tc.tile_pool(name="dgps", bufs=1, space=bass.MemorySpace.PSUM)
        as dgps,
        tc.tile_pool(name="sqp", bufs=2) as sqp,
    ):
        dgp = dgps.tile([P, NB], F32)
        # 1024-wide dma chunks: x (sync) / xyz (gpsimd) -> add(bf16),
        # then per-512-chunk q/diag/v matmuls and incremental q2 dup
        for jc in range(MC // 2):
            for h in range(2):
                xin = zpool.tile([P, 1024], F32, tag="xin")
                nc.sync.dma_start(
                    xin[:],
                    x_d[h * P:(h + 1) * P, jc * 1024:(jc + 1) * 1024])
                zin = zpool.tile([P, 1024], F32, tag="zin")
                nc.sync.dma_start(
                    zin[:],
                    xyz_d[h * P:(h + 1) * P, jc * 1024:(jc + 1) * 1024])
                for u in range(2):
                    c0 = h * N + jc * 1024 + u * 512
                    nc.vector.tensor_add(xs_b[:, c0:c0 + 512],
                                         xin[:, u * 512:(u + 1) * 512],
                                         zin[:, u * 512:(u + 1) * 512])
            for u in range(2):
                j = jc * 2 + u
                # q chunk = wqk @ xs + bqk  -> q2 rows 0:64, bf16
                q_ps = prepps.tile([CQK, 512], F32, tag="q_ps")
                for k in range(2):
                    nc.tensor.matmul(
                        q_ps[:], wqkT[:, k * CQK:(k + 1) * CQK],
                        xs_b[:, k * N + j * 512: k * N + j * 512 + 512],
                        start=(k == 0), stop=(k == 1))
                nc.vector.tensor_scalar_add(q2[0:CQK, j * 512:(j + 1) * 512],
                                            q_ps[:], bqk[:])
                nc.gpsimd.dma_start(q2[CQK:P, j * 512:(j + 1) * 512],
                                    q2[0:CQK, j * 512:(j + 1) * 512])
                # diag: d[n] = ||q_n||^2 straight into [128, NB] layout via
                # per-block transposed N=1 matmuls: sq_chunk^T @ ones64
                sq = sqp.tile([CQK, 512], BF16, tag="sq")
                qs = q2[0:CQK, j * 512:(j + 1) * 512]
                nc.vector.tensor_mul(sq[:], qs, qs)
                for b in range(4):
                    i = j * 4 + b
                    nc.tensor.matmul(dgp[:, i:i + 1], sq[:, b * P:(b + 1) * P],
                                     ones64[:], start=True, stop=True,
                                     skip_group_check=True)
                # v^T tiles for the 4 row blocks covered by this chunk
                for b in range(4):
                    i = j * 4 + b
                    v_ps = prepps.tile([P, C], F32, tag="v_ps")
                    for k in range(2):
                        nc.tensor.matmul(
                            v_ps[:],
                            xs_b[:, k * N + i * P: k * N + i * P + P],
                            wvT[:, k * C:(k + 1) * C],
                            start=(k == 0), stop=(k == 1))
                    nc.scalar.activation(vT[:, i * C:(i + 1) * C], v_ps[:],
                                         mybir.ActivationFunctionType.Copy)
        nc.vector.tensor_scalar_mul(negdiag[:], dgp[:], -1.0)

    # PE clock warm-up: the HAM governor only unthrottles (1.2 -> 2.4 GHz)
    # after a fully-busy 3.4us activity window.  Pass-1's ~85% duty cycle
    # can miss that forever (observed: whole pass 1 cold, +80us).  A solid
    # ~8us back-to-back matmul burst guarantees one fully-busy window.
    with tc.tile_pool(name="warmps", bufs=1,
                      space=bass.MemorySpace.PSUM) as warmps:
        wtile = warmps.tile([CQK, 512], F32)
        for _ in range(16):
            nc.tensor.matmul(wtile[:], wqkT[:, 0:CQK], xs_b[:, 0:512],
                             start=True, stop=True, skip_group_check=True)

    # ---- pass 1: rowsums of exp(E - diag) ----
    # Sinks for the first NCACHE row blocks stay resident in SBUF; pass 2
    # reuses them (scaled by invrs) instead of recomputing E + exp.
    NCACHE = 14
    acache = big.tile([P, NCACHE * N], BF16)
    SW = 2048                      # strip width
    SPB = N // SW                  # strips per block (2)
    CPS = SW // 512                # 512-chunks per strip (4)
    with (
        tc.tile_pool(name="p1ps", bufs=2, space=bass.MemorySpace.PSUM) as p1ps,
        tc.tile_pool(name="p1sc", bufs=1) as p1sc,
    ):
        for i in range(NB):
            for s in range(SPB):
                estrip = p1ps.tile([P, SW], F32, tag="estrip")
                for jj in range(CPS):
                    m0 = s * SW + jj * 512
                    qrow = CQK if jj % 2 == 1 else 0
                    nc.tensor.matmul(
                        estrip[:, jj * 512:(jj + 1) * 512],
                        q2[qrow:qrow + CQK, i * P:(i + 1) * P],
                        q2[qrow:qrow + CQK, m0:m0 + 512],
                        start=True, stop=True)
                if i < NCACHE:
                    sink = acache[:, i * N + s * SW: i * N + (s + 1) * SW]
                else:
                    sink_t = p1sc.tile([P, SW], BF16, tag="sink")
                    sink = sink_t[:]
                nc.scalar.activation(
                    sink, estrip[:], mybir.ActivationFunctionType.Exp,
                    bias=negdiag[:, i:i + 1],
                    accum_out=rs_acc[:, i * SPB + s: i * SPB + s + 1])

    # bias2 = negdiag - ln(rowsum); invrs for the cached-block path
    rs_sum = stats.tile([P, NB], F32)
    nc.vector.tensor_add(rs_sum[:], rs_acc[:, 0:2 * NB:2],
                         rs_acc[:, 1:2 * NB:2])
    lnrs = stats.tile([P, NB], F32)
    nc.scalar.activation(lnrs[:], rs_sum[:], mybir.ActivationFunctionType.Ln)
    nc.vector.tensor_sub(bias2[:], negdiag[:], lnrs[:])
    invrs = stats.tile([P, NB], F32)
    nc.vector.reciprocal(invrs[:], rs_sum[:])

    # ---- pass 2: E -> exp -> x_r accumulation + colsum; tails pipelined ----
    with (
        tc.tile_pool(name="e2ps", bufs=2, space=bass.MemorySpace.PSUM) as e2ps,
        tc.tile_pool(name="xrps", bufs=2, space=bass.MemorySpace.PSUM) as xrps,
        tc.tile_pool(name="tpsp", bufs=1, space=bass.MemorySpace.PSUM) as tpsp,
        tc.tile_pool(name="csps", bufs=1, space=bass.MemorySpace.PSUM) as csps,
        tc.tile_pool(name="a2p", bufs=7) as a2p,
        tc.tile_pool(name="tails", bufs=1) as tails,
        tc.tile_pool(name="csacc", bufs=1) as csacc,
    ):
        def emit_pair(j, p):
            """a2 tiles for row blocks (2p, 2p+1).

            Cached blocks: DVE-scale the resident pass-1 sink by invrs.
            Uncached: E matmuls on opposite PE row halves (they execute
            concurrently) followed by ACT exps."""
            ia, ib = 2 * p, 2 * p + 1
            if ib < NCACHE:
                a2a = a2p.tile([P, 512], BF16, tag="a2")
                nc.vector.tensor_scalar_mul(
                    a2a[:], acache[:, ia * N + j * 512: ia * N + j * 512 + 512],
                    invrs[:, ia:ia + 1])
                a2b = a2p.tile([P, 512], BF16, tag="a2")
                nc.vector.tensor_scalar_mul(
                    a2b[:], acache[:, ib * N + j * 512: ib * N + j * 512 + 512],
                    invrs[:, ib:ib + 1])
                return (ia, a2a, ib, a2b)
            ea = e2ps.tile([P, 512], F32, tag="e2strip")
            nc.tensor.matmul(ea[:], q2[0:CQK, ia * P:(ia + 1) * P],
                             q2[0:CQK, j * 512:(j + 1) * 512],
                             start=True, stop=True)
            eb = e2ps.tile([P, 512], F32, tag="e2strip")
            nc.tensor.matmul(eb[:], q2[CQK:P, ib * P:(ib + 1) * P],
                             q2[CQK:P, j * 512:(j + 1) * 512],
                             start=True, stop=True)
            a2a = a2p.tile([P, 512], BF16, tag="a2")
            nc.scalar.activation(a2a[:], ea[:],
                                 mybir.ActivationFunctionType.Exp,
                                 bias=bias2[:, ia:ia + 1])
            a2b = a2p.tile([P, 512], BF16, tag="a2")
            nc.scalar.activation(a2b[:], eb[:],
                                 mybir.ActivationFunctionType.Exp,
                                 bias=bias2[:, ib:ib + 1])
            return (ia, a2a, ib, a2b)

        def emit_accum(state, pair):
            ia, a2a, ib, a2b = pair
            for (i, a2) in ((ia, a2a), (ib, a2b)):
                first, last = (i == 0), (i == NB - 1)
                for h in range(2):
                    nc.tensor.matmul(
                        state["xr"][h][:],
                        vT[:, i * C + h * P: i * C + h * P + P],
                        a2[:], start=first, stop=last)
            if ib < NCACHE:
                # cached stretch: PE has room for the colsum here
                nc.tensor.matmul(state["cs"][:], ones128[:], a2a[:],
                                 start=(ia == 0), stop=False,
                                 skip_group_check=True)
                nc.tensor.matmul(state["cs"][:], ones128[:], a2b[:],
                                 start=False, stop=False,
                                 skip_group_check=True)
            else:
                # uncached stretch is PE-bound; colsum partials accumulate
                # on the (idle-here) DVE, folded back in tail_a
                if ia == NCACHE:
                    nc.vector.tensor_copy(state["cacc"][:], a2a[:])
                else:
                    nc.vector.tensor_add(state["cacc"][:], state["cacc"][:],
                                         a2a[:])
                nc.vector.tensor_add(state["cacc"][:], state["cacc"][:],
                                     a2b[:])

        # -- tail of chunk j, emitted in 3 slices spread over chunk j+1 --
        def tail_a(j, state):
            nc.tensor.matmul(state["cs"][:], ones_col[:], state["cacc"][:],
                             start=False, stop=True, skip_group_check=True)
            # invcs on the DVE: the ACT Ln/Exp route forced activation
            # table reloads (2 per chunk, ~24us measured) mid pass-2
            cs_eps = tails.tile([1, 512], F32, tag="lncs")
            nc.vector.tensor_scalar_add(cs_eps[:], state["cs"][:], 1e-9)
            invcs = tails.tile([1, 512], F32, tag="invcs")
            nc.vector.reciprocal(invcs[:], cs_eps[:])
            invcs_r = tails.tile([1, 512], F32R, tag="invcs_r")
            nc.vector.tensor_copy(invcs_r[:], invcs[:])
            state["invcs_r"] = invcs_r

        def tail_b(j, state):
            bc_ps = e2ps.tile([P, 512], F32, tag="e2strip", name=f"bc_{j}")
            nc.tensor.matmul(bc_ps[:], ones_row[:], state["invcs_r"][:],
                             start=True, stop=True)
            invcs_bc = tails.tile([P, 512], F32, tag="invcs_bc")
            nc.vector.tensor_copy(invcs_bc[:], bc_ps[:])
            ys = []
            for h in range(2):
                tmp = tails.tile([P, 512], F32, tag=f"tmp{h}")
                nc.vector.tensor_mul(tmp[:], state["xr"][h][:], invcs_bc[:])
                y_h = tails.tile([P, 512], BF16, tag=f"y{h}")
                nc.vector.tensor_sub(
                    y_h[:], xs_b[:, h * N + j * 512: h * N + j * 512 + 512],
                    tmp[:])
                ys.append(y_h)
            state["ys"] = ys

        def tail_c(j, state):
            for ho in range(2):
                t_ps = tpsp.tile([P, 512], F32, tag="tps", name=f"tps{ho}_{j}")
                for k in range(2):
                    nc.tensor.matmul(
                        t_ps[:],
                        wtT[:, k * C + ho * P: k * C + ho * P + P],
                        state["ys"][k][:], start=(k == 0), stop=(k == 1))
                bn = tails.tile([P, 512], F32, tag=f"bn{ho}")
                nc.vector.tensor_scalar(bn[:], t_ps[:], g_t[:, ho:ho + 1],
                                        bp_t[:, ho:ho + 1],
                                        mybir.AluOpType.mult,
                                        mybir.AluOpType.add)
                lr = tails.tile([P, 512], F32, tag=f"lr{ho}")
                nc.vector.scalar_tensor_tensor(lr[:], bn[:], 0.2, bn[:],
                                               mybir.AluOpType.mult,
                                               mybir.AluOpType.max)
                o_t = tails.tile([P, 512], F32, tag=f"o{ho}")
                nc.vector.tensor_add(
                    o_t[:], lr[:],
                    xs_b[:, ho * N + j * 512: ho * N + j * 512 + 512])
                nc.sync.dma_start(
                    out_d[ho * P:(ho + 1) * P, j * 512:(j + 1) * 512],
                    o_t[:])

        prev = None
        for j in range(MC):
            state = {
                "cs": csps.tile([1, 512], F32, tag="cs", name=f"cs_{j}"),
                "cacc": csacc.tile([P, 512], F32R, tag="cacc",
                                   name=f"cacc_{j}"),

                "xr": [xrps.tile([P, 512], F32, tag=f"xr{h}",
                                 name=f"xr{h}_{j}")
                       for h in range(2)],
            }
            pair_prev = emit_pair(j, 0)
            for p in range(1, NPAIR):
                pair_cur = emit_pair(j, p)
                if prev is not None:
                    if p == 1:
                        tail_a(j - 1, prev)
                    elif p == 3:
                        tail_b(j - 1, prev)
                    elif p == 5:
                        tail_c(j - 1, prev)
                emit_accum(state, pair_prev)
                pair_prev = pair_cur
            emit_accum(state, pair_prev)
            prev = state
        tail_a(MC - 1, prev)
        tail_b(MC - 1, prev)
        tail_c(MC - 1, prev)


# ---------------------------------------------------------------------------
# host-side wrapper
# ---------------------------------------------------------------------------
_NC_CACHE = {}


def _get_nc(N=4096):
    if N not in _NC_CACHE:
        _NC_CACHE[N] = build_kernel(N=N)
    return _NC_CACHE[N]


def host_prep(wqk, bqk, wv, bv, wt, bt, bn_gamma, bn_beta, bn_mean, bn_var):
    bf16 = mybir.dt.np(BF16)
    wqk = np.asarray(wqk, np.float32)
    wv = np.asarray(wv, np.float32)
    wt = np.asarray(wt, np.float32)
    g = (np.asarray(bn_gamma, np.float32)
         / np.sqrt(np.asarray(bn_var, np.float32) + BN_EPS))
    bp = np.asarray(bn_beta, np.float32) - np.asarray(bn_mean, np.float32) * g
    btp = np.asarray(bt, np.float32) - wt @ np.asarray(bv, np.float32)
    bp_eff = btp * g + bp
    return {
        "wqkT": np.ascontiguousarray(wqk.T).astype(bf16),
        "wvT": np.ascontiguousarray(wv.T).astype(bf16),
        "wtT": np.ascontiguousarray(wt.T).astype(bf16),
        "bqk": np.asarray(bqk, np.float32).reshape(CQK, 1),
        "g": g.reshape(C, 1),
        "bp": bp_eff.reshape(C, 1),
    }


def kernel(x, xyz, wqk, bqk, wv, bv, wt, bt, bn_gamma, bn_beta, bn_mean,
           bn_var, _profile=False):
    from concourse.bass_utils import run_bass_kernel_spmd

    x = np.asarray(x, np.float32)
    xyz = np.asarray(xyz, np.float32)
    B, Cc, N = x.shape
    assert Cc == C and B == 8
    nc = _get_nc(N)
    wmap = host_prep(wqk, bqk, wv, bv, wt, bt, bn_gamma, bn_beta, bn_mean,
                     bn_var)
    in_maps = [
        {"x": np.ascontiguousarray(x[b]),
         "xyz": np.ascontiguousarray(xyz[b]), **wmap}
        for b in range(B)
    ]
    res = run_bass_kernel_spmd(nc, in_maps, list(range(8)), trace=_profile)
    out = np.stack([res.results[b]["out"] for b in range(B)], axis=0)
    if _profile:
        return out, res
    return out
